# revision 5
# baseline (speedup 1.0000x reference)
import sys

sys.path.insert(0, "/opt/trn_rl_repo")

from contextlib import ExitStack

import ml_dtypes
import numpy as np

from concourse import bass, mybir, tile
from concourse.bass_utils import run_bass_kernel_spmd
from concourse.vector_clock import ScopedClock


def _patched_drain_and_barrier(self, tick_clock, wait_clock):
    # Workaround: this compiler rejects a drain carrying >1 sem wait
    # ([NCC_INLA001]); split extra waits onto single-wait nops.
    drain_inst = self.nc.sync.drain()
    wait_clock.add_sem_waits(
        drain_inst.ins, ScopedClock({None: tick_clock.global_clock})
    )
    si = drain_inst.ins.sync_info
    waits = list(si.on_wait) if si and si.on_wait else []
    if len(waits) > 1:
        drain_inst.ins.sync_info = mybir.SyncInfo(
            on_wait=[waits[0]], on_update=list(si.on_update or [])
        )
        for w in waits[1:]:
            nop = self.nc.sync.nop(nofuse=True)
            nop.ins.sync_info = mybir.SyncInfo(on_wait=[w], on_update=[])
    self.nc.all_engine_barrier()
    popped = self.nc._tile_sem_poison_stack.pop()
    assert popped is self._sem_poison
    self.nc.clear_and_free_semaphores(list(self.sems.allocated().values()))
    self.nc.all_engine_barrier()


tile.TileContext._drain_and_barrier = _patched_drain_and_barrier


def _split_excess_waits(nc, limit=1):
    # Workaround: this compiler allows only one sem wait on several
    # instruction encodings; move extra waits onto same-engine nops.
    eng_map = {
        mybir.EngineType.PE: nc.tensor,
        mybir.EngineType.Activation: nc.scalar,
        mybir.EngineType.DVE: nc.vector,
        mybir.EngineType.Pool: nc.gpsimd,
        mybir.EngineType.SP: nc.sync,
    }
    for blk in nc.cur_f.blocks:
        orig = list(blk.instructions)
        out = []
        for ins in orig:
            si = ins.sync_info
            waits = list(si.on_wait) if si and si.on_wait else []
            eng = eng_map.get(ins.engine)
            if len(waits) > limit and eng is not None:
                extra, keep = waits[:-limit], waits[-limit:]
                for w in extra:
                    nop = eng.nop(nofuse=True).ins
                    tail = nc.cur_f.blocks[-1].instructions
                    assert tail[-1] is nop
                    tail.pop()
                    nop.sync_info = mybir.SyncInfo(on_wait=[w], on_update=[])
                    out.append(nop)
                ins.sync_info = mybir.SyncInfo(
                    on_wait=keep, on_update=list(si.on_update or [])
                )
            out.append(ins)
        blk.instructions[:] = out

bf16 = ml_dtypes.bfloat16
BF = bass.mybir.dt.bfloat16
F32 = bass.mybir.dt.float32
AF = mybir.ActivationFunctionType
ALU = mybir.AluOpType

B, S, E, H, D = 2, 2048, 2048, 16, 128
BS = B * S
NCORES = 8
HPC = H // NCORES  # heads per core
DC = HPC * D  # per-core head-dim width (256)
SCALE = 1.0 / float(np.sqrt(D))
NEG = -1.0e9

TRACE = False
LAST_RESULTS = None
_NC_CACHE = None


def _build():
    nc = bass.Bass()
    xT = nc.declare_dram_parameter("xT", (E, BS), BF, isOutput=False)
    wqT = nc.declare_dram_parameter("wqT", (E, DC), BF, isOutput=False)
    wkT = nc.declare_dram_parameter("wkT", (E, DC), BF, isOutput=False)
    wvT = nc.declare_dram_parameter("wvT", (E, DC), BF, isOutput=False)
    woT = nc.declare_dram_parameter("woT", (DC, E), BF, isOutput=False)
    bqd = nc.declare_dram_parameter("bq", (DC, 1), F32, isOutput=False)
    bkd = nc.declare_dram_parameter("bk", (DC, 1), F32, isOutput=False)
    mskd = nc.declare_dram_parameter("mask", (512, 512), BF, isOutput=False)
    idnd = nc.declare_dram_parameter("ident", (128, 128), BF, isOutput=False)
    onkd = nc.declare_dram_parameter("onesk", (128, 1), BF, isOutput=False)
    on1d = nc.declare_dram_parameter("ones1", (1, 128), F32, isOutput=False)
    yd = nc.declare_dram_parameter("y", (BS, E), F32, isOutput=True)

    with ExitStack() as ctx:
        tc = ctx.enter_context(tile.TileContext(nc))
        wp = ctx.enter_context(tc.tile_pool(name="wp", bufs=1))
        bp = ctx.enter_context(tc.tile_pool(name="bp", bufs=1))
        pp = ctx.enter_context(tc.tile_pool(name="pp", bufs=17))
        dp = ctx.enter_context(tc.tile_pool(name="dp", bufs=2))
        yp = ctx.enter_context(tc.tile_pool(name="yp", bufs=3))
        ps = ctx.enter_context(tc.tile_pool(name="ps", bufs=1, space="PSUM"))

        wq_sb = wp.tile([128, 16, DC], BF)
        wk_sb = wp.tile([128, 16, DC], BF)
        wv_sb = wp.tile([128, 16, DC], BF)
        wo_sb = wp.tile([128, HPC, E], BF)
        bq_sb = wp.tile([128, HPC, 1], F32)
        bk_sb = wp.tile([128, HPC, 1], F32)
        msk_sb = wp.tile([128, 4, 512], BF)
        idn_sb = wp.tile([128, 128], BF)
        onk_sb = wp.tile([128, 1], BF)
        on1_sb = wp.tile([1, 128], F32)

        for t in range(16):
            nc.sync.dma_start(wq_sb[:, t, :], wqT[t * 128 : (t + 1) * 128, :])
            nc.sync.dma_start(wk_sb[:, t, :], wkT[t * 128 : (t + 1) * 128, :])
            nc.sync.dma_start(wv_sb[:, t, :], wvT[t * 128 : (t + 1) * 128, :])
        for h in range(HPC):
            nc.sync.dma_start(wo_sb[:, h, :], woT[h * 128 : (h + 1) * 128, :])
            nc.sync.dma_start(bq_sb[:, h, :], bqd[h * 128 : (h + 1) * 128, :])
            nc.sync.dma_start(bk_sb[:, h, :], bkd[h * 128 : (h + 1) * 128, :])
        for r in range(4):
            nc.sync.dma_start(msk_sb[:, r, :], mskd[r * 128 : (r + 1) * 128, :])
        nc.sync.dma_start(idn_sb[:], idnd[:])
        nc.sync.dma_start(onk_sb[:], onkd[:])
        nc.sync.dma_start(on1_sb[:], on1d[:])

        for b in range(B):
            s0 = b * S
            x_sb = bp.tile([128, 16, S], BF)
            for t in range(16):
                nc.sync.dma_start(x_sb[:, t, :], xT[t * 128 : (t + 1) * 128, s0 : s0 + S])
            qT_sb = bp.tile([128, HPC, S], BF)
            kT_sb = bp.tile([128, HPC, S], BF)
            v_sb = bp.tile([128, 16, DC], BF)
            ctx_sb = bp.tile([128, HPC, S], F32)
            ctxN_sb = bp.tile([128, HPC, S], BF)

            # --- q/k/v projections (contract E in 16 chunks of 128) ---
            for m in range(HPC):
                for j in range(4):
                    js = slice(j * 512, (j + 1) * 512)
                    q_ps = ps.tile([128, 512], F32, tag="pr", bufs=2)
                    for t in range(16):
                        nc.tensor.matmul(
                            q_ps[:],
                            wq_sb[:, t, m * 128 : (m + 1) * 128],
                            x_sb[:, t, js],
                            start=(t == 0),
                            stop=(t == 15),
                        )
                    nc.scalar.activation(
                        qT_sb[:, m, js], q_ps[:], AF.Identity, bias=bq_sb[:, m, :]
                    )
                    k_ps = ps.tile([128, 512], F32, tag="pr", bufs=2)
                    for t in range(16):
                        nc.tensor.matmul(
                            k_ps[:],
                            wk_sb[:, t, m * 128 : (m + 1) * 128],
                            x_sb[:, t, js],
                            start=(t == 0),
                            stop=(t == 15),
                        )
                    nc.scalar.activation(
                        kT_sb[:, m, js], k_ps[:], AF.Identity, bias=bk_sb[:, m, :]
                    )
            for si in range(16):
                v_ps = ps.tile([128, DC], F32, tag="pr", bufs=2)
                for t in range(16):
                    nc.tensor.matmul(
                        v_ps[:],
                        x_sb[:, t, si * 128 : (si + 1) * 128],
                        wv_sb[:, t, :],
                        start=(t == 0),
                        stop=(t == 15),
                    )
                nc.vector.tensor_copy(v_sb[:, si, :], v_ps[:])

            # --- causal attention, scores kept transposed [k, q] ---
            for h in range(HPC):
                hd = slice(h * 128, (h + 1) * 128)
                for qb in range(4):
                    qs = slice(qb * 512, (qb + 1) * 512)
                    kmax = 4 * qb + 4
                    pts = []
                    for kc in range(kmax):
                        sc_ps = ps.tile([128, 512], F32, tag="sc", bufs=3)
                        diag = kc - 4 * qb
                        nc.tensor.matmul(
                            sc_ps[:],
                            kT_sb[:, h, kc * 128 : (kc + 1) * 128],
                            qT_sb[:, h, qs],
                            start=True,
                            stop=(diag < 0),
                        )
                        if diag >= 0:
                            nc.tensor.matmul(
                                sc_ps[:],
                                idn_sb[:],
                                msk_sb[:, diag, :],
                                start=False,
                                stop=True,
                            )
                        p_t = pp.tile([128, 512], BF)
                        nc.scalar.activation(p_t[:], sc_ps[:], AF.Exp)
                        pts.append(p_t)
                    den_ps = ps.tile([1, 512], F32, tag="dn", bufs=1)
                    for kc in range(kmax):
                        nc.tensor.matmul(
                            den_ps[:],
                            onk_sb[:],
                            pts[kc][:],
                            start=(kc == 0),
                            stop=(kc == kmax - 1),
                        )
                    ctx_ps = ps.tile([128, 512], F32, tag="cx", bufs=1)
                    for kc in range(kmax):
                        nc.tensor.matmul(
                            ctx_ps[:],
                            v_sb[:, kc, hd],
                            pts[kc][:],
                            start=(kc == 0),
                            stop=(kc == kmax - 1),
                        )
                    den_sb = dp.tile([1, 512], F32, tag="den", bufs=2)
                    nc.scalar.copy(den_sb[:], den_ps[:])
                    rec_sb = dp.tile([1, 512], F32, tag="rec", bufs=2)
                    nc.vector.reciprocal(rec_sb[:], den_sb[:])
                    bc_ps = ps.tile([128, 512], F32, tag="bc", bufs=1)
                    nc.tensor.matmul(
                        bc_ps[:], on1_sb[:], rec_sb[:], start=True, stop=True
                    )
                    nc.scalar.copy(ctx_sb[:, h, qs], ctx_ps[:])
                    nc.vector.tensor_tensor(
                        ctxN_sb[:, h, qs], ctx_sb[:, h, qs], bc_ps[:], ALU.mult
                    )

            # --- output projection (contract per-core d=256 in 2 head chunks) ---
            for qc in range(16):
                for eb in range(4):
                    y_ps = ps.tile([128, 512], F32, tag="pr", bufs=2)
                    nc.tensor.matmul(
                        y_ps[:],
                        ctxN_sb[:, 0, qc * 128 : (qc + 1) * 128],
                        wo_sb[:, 0, eb * 512 : (eb + 1) * 512],
                        start=True,
                        stop=False,
                    )
                    nc.tensor.matmul(
                        y_ps[:],
                        ctxN_sb[:, 1, qc * 128 : (qc + 1) * 128],
                        wo_sb[:, 1, eb * 512 : (eb + 1) * 512],
                        start=False,
                        stop=True,
                    )
                    y_t = yp.tile([128, 512], F32)
                    nc.vector.tensor_copy(y_t[:], y_ps[:])
                    nc.sync.dma_start(
                        yd[s0 + qc * 128 : s0 + (qc + 1) * 128, eb * 512 : (eb + 1) * 512],
                        y_t[:],
                    )
    _split_excess_waits(nc)
    return nc


def _mask_np():
    m = np.zeros((4, 128, 512), np.float32)
    kk = np.arange(128)[:, None]
    qq = np.arange(512)[None, :]
    for r in range(4):
        m[r] = np.where(kk + 128 * r > qq, NEG, 0.0)
    return m.reshape(512, 512).astype(bf16)


def kernel(**inputs):
    global LAST_RESULTS, _NC_CACHE
    x = np.asarray(inputs["x"], np.float32)
    Wq = np.asarray(inputs["Wq"], np.float32)
    bq = np.asarray(inputs["bq"], np.float32)
    Wk = np.asarray(inputs["Wk"], np.float32)
    bk = np.asarray(inputs["bk"], np.float32)
    Wv = np.asarray(inputs["Wv"], np.float32)
    bv = np.asarray(inputs["bv"], np.float32)
    Wo = np.asarray(inputs["Wo"], np.float32)
    bo = np.asarray(inputs["bo"], np.float32)

    xT = np.ascontiguousarray(x.reshape(BS, E).T).astype(bf16)
    mask = _mask_np()
    ident = np.eye(128, dtype=np.float32).astype(bf16)
    onesk = np.ones((128, 1), bf16)
    ones1 = np.ones((1, 128), np.float32)

    in_maps = []
    for c in range(NCORES):
        dsl = slice(c * DC, (c + 1) * DC)
        in_maps.append(
            {
                "xT": xT,
                "wqT": np.ascontiguousarray(Wq[dsl].T * SCALE).astype(bf16),
                "wkT": np.ascontiguousarray(Wk[dsl].T).astype(bf16),
                "wvT": np.ascontiguousarray(Wv[dsl].T).astype(bf16),
                "woT": np.ascontiguousarray(Wo[:, dsl].T).astype(bf16),
                "bq": np.ascontiguousarray((bq[dsl] * SCALE).reshape(DC, 1)),
                "bk": np.ascontiguousarray(bk[dsl].reshape(DC, 1)),
                "mask": mask,
                "ident": ident,
                "onesk": onesk,
                "ones1": ones1,
            }
        )

    if _NC_CACHE is None:
        _NC_CACHE = _build()
    res = run_bass_kernel_spmd(_NC_CACHE, in_maps, core_ids=list(range(NCORES)), trace=TRACE)
    LAST_RESULTS = res

    acc = None
    for r in res.results:
        yc = np.asarray(r["y"], np.float32)
        acc = yc if acc is None else acc + yc
    bo_eff = bo + bv @ Wo.T
    acc += bo_eff[None, :]
    return acc.reshape(B, S, E).astype(np.float32)


# revision 16
# speedup vs baseline: 1.4398x; 1.4398x over previous
import sys

sys.path.insert(0, "/opt/trn_rl_repo")

from contextlib import ExitStack

import ml_dtypes
import numpy as np

from concourse import bass, mybir, tile
from concourse.bass_utils import run_bass_kernel_spmd
from concourse.vector_clock import ScopedClock


def _patched_drain_and_barrier(self, tick_clock, wait_clock):
    # Workaround: this compiler rejects a drain carrying >1 sem wait
    # ([NCC_INLA001]); split extra waits onto single-wait nops.
    drain_inst = self.nc.sync.drain()
    wait_clock.add_sem_waits(
        drain_inst.ins, ScopedClock({None: tick_clock.global_clock})
    )
    si = drain_inst.ins.sync_info
    waits = list(si.on_wait) if si and si.on_wait else []
    if len(waits) > 1:
        drain_inst.ins.sync_info = mybir.SyncInfo(
            on_wait=[waits[0]], on_update=list(si.on_update or [])
        )
        for w in waits[1:]:
            nop = self.nc.sync.nop(nofuse=True)
            nop.ins.sync_info = mybir.SyncInfo(on_wait=[w], on_update=[])
    self.nc.all_engine_barrier()
    popped = self.nc._tile_sem_poison_stack.pop()
    assert popped is self._sem_poison
    self.nc.clear_and_free_semaphores(list(self.sems.allocated().values()))
    self.nc.all_engine_barrier()


tile.TileContext._drain_and_barrier = _patched_drain_and_barrier


def _split_excess_waits(nc, limit=1):
    # Workaround: this compiler allows only one sem wait on several
    # instruction encodings; move extra waits onto same-engine nops.
    eng_map = {
        mybir.EngineType.PE: nc.tensor,
        mybir.EngineType.Activation: nc.scalar,
        mybir.EngineType.DVE: nc.vector,
        mybir.EngineType.Pool: nc.gpsimd,
        mybir.EngineType.SP: nc.sync,
    }
    for blk in nc.cur_f.blocks:
        orig = list(blk.instructions)
        out = []
        for ins in orig:
            si = ins.sync_info
            waits = list(si.on_wait) if si and si.on_wait else []
            eng = eng_map.get(ins.engine)
            if len(waits) > limit and eng is not None:
                extra, keep = waits[:-limit], waits[-limit:]
                for w in extra:
                    nop = eng.nop(nofuse=True).ins
                    tail = nc.cur_f.blocks[-1].instructions
                    assert tail[-1] is nop
                    tail.pop()
                    nop.sync_info = mybir.SyncInfo(on_wait=[w], on_update=[])
                    out.append(nop)
                ins.sync_info = mybir.SyncInfo(
                    on_wait=keep, on_update=list(si.on_update or [])
                )
            out.append(ins)
        blk.instructions[:] = out

bf16 = ml_dtypes.bfloat16
BF = bass.mybir.dt.bfloat16
F32 = bass.mybir.dt.float32
AF = mybir.ActivationFunctionType
ALU = mybir.AluOpType

B, S, E, H, D = 2, 2048, 2048, 16, 128
BS = B * S
NCORES = 8
HPC = H // NCORES  # heads per core
DC = HPC * D  # per-core head-dim width (256)
SCALE = 1.0 / float(np.sqrt(D))
NEG = -1.0e9

TRACE = False
LAST_RESULTS = None
_NC_CACHE = None


def _build():
    nc = bass.Bass()
    xT = nc.declare_dram_parameter("xT", (E, BS), BF, isOutput=False)
    wqT = nc.declare_dram_parameter("wqT", (E, DC), BF, isOutput=False)
    wkT = nc.declare_dram_parameter("wkT", (E, DC), BF, isOutput=False)
    wvT = nc.declare_dram_parameter("wvT", (E, DC), BF, isOutput=False)
    woT = nc.declare_dram_parameter("woT", (DC, E), BF, isOutput=False)
    bqd = nc.declare_dram_parameter("bq", (DC, 1), F32, isOutput=False)
    bkd = nc.declare_dram_parameter("bk", (DC, 1), F32, isOutput=False)
    mskd = nc.declare_dram_parameter("mask", (512, 512), BF, isOutput=False)
    onkd = nc.declare_dram_parameter("onesk", (128, 128), BF, isOutput=False)
    yd = nc.declare_dram_parameter("y", (BS, E), F32, isOutput=True)

    with ExitStack() as ctx:
        tc = ctx.enter_context(tile.TileContext(nc))
        wp = ctx.enter_context(tc.tile_pool(name="wp", bufs=1))
        bp = ctx.enter_context(tc.tile_pool(name="bp", bufs=1))
        pp = ctx.enter_context(tc.tile_pool(name="pp", bufs=17))
        dp = ctx.enter_context(tc.tile_pool(name="dp", bufs=2))
        yp = ctx.enter_context(tc.tile_pool(name="yp", bufs=3))
        ps = ctx.enter_context(tc.tile_pool(name="ps", bufs=1, space="PSUM"))

        wq_sb = wp.tile([128, 16, DC], BF)
        wk_sb = wp.tile([128, 16, DC], BF)
        wv_sb = wp.tile([128, 16, DC], BF)
        wo_sb = wp.tile([128, HPC, E], BF)
        bq_sb = wp.tile([128, HPC, 1], F32)
        bk_sb = wp.tile([128, HPC, 1], F32)
        msk_sb = wp.tile([128, 4, 512], BF)
        onk_sb = wp.tile([128, 128], BF)

        for t in range(16):
            nc.sync.dma_start(wq_sb[:, t, :], wqT[t * 128 : (t + 1) * 128, :])
            nc.sync.dma_start(wk_sb[:, t, :], wkT[t * 128 : (t + 1) * 128, :])
            nc.sync.dma_start(wv_sb[:, t, :], wvT[t * 128 : (t + 1) * 128, :])
        for h in range(HPC):
            nc.sync.dma_start(wo_sb[:, h, :], woT[h * 128 : (h + 1) * 128, :])
            nc.sync.dma_start(bq_sb[:, h, :], bqd[h * 128 : (h + 1) * 128, :])
            nc.sync.dma_start(bk_sb[:, h, :], bkd[h * 128 : (h + 1) * 128, :])
        for r in range(4):
            nc.sync.dma_start(msk_sb[:, r, :], mskd[r * 128 : (r + 1) * 128, :])
        nc.sync.dma_start(onk_sb[:], onkd[:])

        for b in range(B):
            s0 = b * S
            x_sb = bp.tile([128, 16, S], BF)
            for t in range(16):
                nc.sync.dma_start(x_sb[:, t, :], xT[t * 128 : (t + 1) * 128, s0 : s0 + S])
            qT_sb = bp.tile([128, HPC, S], BF)
            kT_sb = bp.tile([128, HPC, S], BF)
            v_sb = bp.tile([128, 16, DC], BF)
            ctxN_sb = bp.tile([128, HPC, S], BF)

            # --- q/k/v projections (contract E in 16 chunks of 128) ---
            for m in range(HPC):
                for j in range(4):
                    js = slice(j * 512, (j + 1) * 512)
                    q_ps = ps.tile([128, 512], F32, tag="pr", bufs=2)
                    for t in range(16):
                        nc.tensor.matmul(
                            q_ps[:],
                            wq_sb[:, t, m * 128 : (m + 1) * 128],
                            x_sb[:, t, js],
                            start=(t == 0),
                            stop=(t == 15),
                        )
                    nc.scalar.activation(
                        qT_sb[:, m, js], q_ps[:], AF.Identity, bias=bq_sb[:, m, :]
                    )
                    k_ps = ps.tile([128, 512], F32, tag="pr", bufs=2)
                    for t in range(16):
                        nc.tensor.matmul(
                            k_ps[:],
                            wk_sb[:, t, m * 128 : (m + 1) * 128],
                            x_sb[:, t, js],
                            start=(t == 0),
                            stop=(t == 15),
                        )
                    nc.scalar.activation(
                        kT_sb[:, m, js], k_ps[:], AF.Identity, bias=bk_sb[:, m, :]
                    )
            for si in range(16):
                v_ps = ps.tile([128, DC], F32, tag="pr", bufs=2)
                for t in range(16):
                    nc.tensor.matmul(
                        v_ps[:],
                        x_sb[:, t, si * 128 : (si + 1) * 128],
                        wv_sb[:, t, :],
                        start=(t == 0),
                        stop=(t == 15),
                    )
                nc.vector.tensor_copy(v_sb[:, si, :], v_ps[:])

            # --- causal attention, scores kept transposed [k, q] ---
            for h in range(HPC):
                hd = slice(h * 128, (h + 1) * 128)
                for qb in range(4):
                    qs = slice(qb * 512, (qb + 1) * 512)
                    kmax = 4 * qb + 4
                    pts = []
                    for kc in range(kmax):
                        sc_ps = ps.tile([128, 512], F32, tag="sc", bufs=3)
                        diag = kc - 4 * qb
                        nc.tensor.matmul(
                            sc_ps[:],
                            kT_sb[:, h, kc * 128 : (kc + 1) * 128],
                            qT_sb[:, h, qs],
                            start=True,
                            stop=True,
                        )
                        p_t = pp.tile([128, 512], BF)
                        nc.scalar.activation(p_t[:], sc_ps[:], AF.Exp)
                        if diag >= 0:
                            nc.vector.tensor_tensor(
                                p_t[:], p_t[:], msk_sb[:, diag, :], ALU.mult
                            )
                        pts.append(p_t)
                    den_ps = ps.tile([128, 512], F32, tag="dn", bufs=1)
                    for kc in range(kmax):
                        nc.tensor.matmul(
                            den_ps[:],
                            onk_sb[:],
                            pts[kc][:],
                            start=(kc == 0),
                            stop=(kc == kmax - 1),
                        )
                    ctx_ps = ps.tile([128, 512], F32, tag="cx", bufs=2)
                    for kc in range(kmax):
                        nc.tensor.matmul(
                            ctx_ps[:],
                            v_sb[:, kc, hd],
                            pts[kc][:],
                            start=(kc == 0),
                            stop=(kc == kmax - 1),
                        )
                    lnd_sb = dp.tile([128, 512], F32, tag="lnd", bufs=2)
                    nc.scalar.activation(lnd_sb[:], den_ps[:], AF.Ln)
                    recb_sb = dp.tile([128, 512], F32, tag="recb", bufs=2)
                    nc.scalar.activation(recb_sb[:], lnd_sb[:], AF.Exp, scale=-1.0)
                    nc.vector.tensor_tensor(
                        ctxN_sb[:, h, qs], ctx_ps[:], recb_sb[:], ALU.mult
                    )

            # --- output projection (contract per-core d=256 in 2 head chunks) ---
            for qc in range(16):
                for eb in range(4):
                    y_ps = ps.tile([128, 512], F32, tag="pr", bufs=2)
                    nc.tensor.matmul(
                        y_ps[:],
                        ctxN_sb[:, 0, qc * 128 : (qc + 1) * 128],
                        wo_sb[:, 0, eb * 512 : (eb + 1) * 512],
                        start=True,
                        stop=False,
                    )
                    nc.tensor.matmul(
                        y_ps[:],
                        ctxN_sb[:, 1, qc * 128 : (qc + 1) * 128],
                        wo_sb[:, 1, eb * 512 : (eb + 1) * 512],
                        start=False,
                        stop=True,
                    )
                    y_t = yp.tile([128, 512], F32)
                    if (qc * 4 + eb) % 2 == 0:
                        nc.vector.tensor_copy(y_t[:], y_ps[:])
                    else:
                        nc.scalar.copy(y_t[:], y_ps[:])
                    nc.sync.dma_start(
                        yd[s0 + qc * 128 : s0 + (qc + 1) * 128, eb * 512 : (eb + 1) * 512],
                        y_t[:],
                    )
    _split_excess_waits(nc)
    return nc


def _mask_np():
    m = np.zeros((4, 128, 512), np.float32)
    kk = np.arange(128)[:, None]
    qq = np.arange(512)[None, :]
    for r in range(4):
        m[r] = np.where(kk + 128 * r > qq, 0.0, 1.0)
    return m.reshape(512, 512).astype(bf16)


def kernel(**inputs):
    global LAST_RESULTS, _NC_CACHE
    x = np.asarray(inputs["x"], np.float32)
    Wq = np.asarray(inputs["Wq"], np.float32)
    bq = np.asarray(inputs["bq"], np.float32)
    Wk = np.asarray(inputs["Wk"], np.float32)
    bk = np.asarray(inputs["bk"], np.float32)
    Wv = np.asarray(inputs["Wv"], np.float32)
    bv = np.asarray(inputs["bv"], np.float32)
    Wo = np.asarray(inputs["Wo"], np.float32)
    bo = np.asarray(inputs["bo"], np.float32)

    xT = np.ascontiguousarray(x.reshape(BS, E).T).astype(bf16)
    mask = _mask_np()
    onesk = np.ones((128, 128), bf16)

    in_maps = []
    for c in range(NCORES):
        dsl = slice(c * DC, (c + 1) * DC)
        in_maps.append(
            {
                "xT": xT,
                "wqT": np.ascontiguousarray(Wq[dsl].T * SCALE).astype(bf16),
                "wkT": np.ascontiguousarray(Wk[dsl].T).astype(bf16),
                "wvT": np.ascontiguousarray(Wv[dsl].T).astype(bf16),
                "woT": np.ascontiguousarray(Wo[:, dsl].T).astype(bf16),
                "bq": np.ascontiguousarray((bq[dsl] * SCALE).reshape(DC, 1)),
                "bk": np.ascontiguousarray(bk[dsl].reshape(DC, 1)),
                "mask": mask,
                "onesk": onesk,
            }
        )

    if _NC_CACHE is None:
        _NC_CACHE = _build()
    res = run_bass_kernel_spmd(_NC_CACHE, in_maps, core_ids=list(range(NCORES)), trace=TRACE)
    LAST_RESULTS = res

    acc = None
    for r in res.results:
        yc = np.asarray(r["y"], np.float32)
        acc = yc if acc is None else acc + yc
    bo_eff = bo + bv @ Wo.T
    acc += bo_eff[None, :]
    return acc.reshape(B, S, E).astype(np.float32)


# revision 17
# speedup vs baseline: 1.5153x; 1.0524x over previous
import sys

sys.path.insert(0, "/opt/trn_rl_repo")

from contextlib import ExitStack

import ml_dtypes
import numpy as np

from concourse import bass, mybir, tile
from concourse.bass_utils import run_bass_kernel_spmd
from concourse.vector_clock import ScopedClock


def _patched_drain_and_barrier(self, tick_clock, wait_clock):
    # Workaround: this compiler rejects a drain carrying >1 sem wait
    # ([NCC_INLA001]); split extra waits onto single-wait nops.
    drain_inst = self.nc.sync.drain()
    wait_clock.add_sem_waits(
        drain_inst.ins, ScopedClock({None: tick_clock.global_clock})
    )
    si = drain_inst.ins.sync_info
    waits = list(si.on_wait) if si and si.on_wait else []
    if len(waits) > 1:
        drain_inst.ins.sync_info = mybir.SyncInfo(
            on_wait=[waits[0]], on_update=list(si.on_update or [])
        )
        for w in waits[1:]:
            nop = self.nc.sync.nop(nofuse=True)
            nop.ins.sync_info = mybir.SyncInfo(on_wait=[w], on_update=[])
    self.nc.all_engine_barrier()
    popped = self.nc._tile_sem_poison_stack.pop()
    assert popped is self._sem_poison
    self.nc.clear_and_free_semaphores(list(self.sems.allocated().values()))
    self.nc.all_engine_barrier()


tile.TileContext._drain_and_barrier = _patched_drain_and_barrier


def _split_excess_waits(nc, limit=1):
    # Workaround: this compiler allows only one sem wait on several
    # instruction encodings; move extra waits onto same-engine nops.
    eng_map = {
        mybir.EngineType.PE: nc.tensor,
        mybir.EngineType.Activation: nc.scalar,
        mybir.EngineType.DVE: nc.vector,
        mybir.EngineType.Pool: nc.gpsimd,
        mybir.EngineType.SP: nc.sync,
    }
    for blk in nc.cur_f.blocks:
        orig = list(blk.instructions)
        out = []
        for ins in orig:
            si = ins.sync_info
            waits = list(si.on_wait) if si and si.on_wait else []
            eng = eng_map.get(ins.engine)
            if len(waits) > limit and eng is not None:
                extra, keep = waits[:-limit], waits[-limit:]
                for w in extra:
                    nop = eng.nop(nofuse=True).ins
                    tail = nc.cur_f.blocks[-1].instructions
                    assert tail[-1] is nop
                    tail.pop()
                    nop.sync_info = mybir.SyncInfo(on_wait=[w], on_update=[])
                    out.append(nop)
                ins.sync_info = mybir.SyncInfo(
                    on_wait=keep, on_update=list(si.on_update or [])
                )
            out.append(ins)
        blk.instructions[:] = out

bf16 = ml_dtypes.bfloat16
BF = bass.mybir.dt.bfloat16
F32 = bass.mybir.dt.float32
AF = mybir.ActivationFunctionType
ALU = mybir.AluOpType

B, S, E, H, D = 2, 2048, 2048, 16, 128
BS = B * S
NCORES = 8
HPC = H // NCORES  # heads per core
DC = HPC * D  # per-core head-dim width (256)
SCALE = 1.0 / float(np.sqrt(D))
NEG = -1.0e9

TRACE = False
LAST_RESULTS = None
_NC_CACHE = None


def _build():
    nc = bass.Bass()
    xT = nc.declare_dram_parameter("xT", (E, BS), BF, isOutput=False)
    wqT = nc.declare_dram_parameter("wqT", (E, DC), BF, isOutput=False)
    wkT = nc.declare_dram_parameter("wkT", (E, DC), BF, isOutput=False)
    wvT = nc.declare_dram_parameter("wvT", (E, DC), BF, isOutput=False)
    woT = nc.declare_dram_parameter("woT", (DC, E), BF, isOutput=False)
    bqd = nc.declare_dram_parameter("bq", (DC, 1), F32, isOutput=False)
    bkd = nc.declare_dram_parameter("bk", (DC, 1), F32, isOutput=False)
    mskd = nc.declare_dram_parameter("mask", (512, 512), BF, isOutput=False)
    onkd = nc.declare_dram_parameter("onesk", (128, 128), BF, isOutput=False)
    yd = nc.declare_dram_parameter("y", (BS, E), F32, isOutput=True)

    with ExitStack() as ctx:
        tc = ctx.enter_context(tile.TileContext(nc))
        wp = ctx.enter_context(tc.tile_pool(name="wp", bufs=1))
        bp = ctx.enter_context(tc.tile_pool(name="bp", bufs=1))
        pp = ctx.enter_context(tc.tile_pool(name="pp", bufs=17))
        dp = ctx.enter_context(tc.tile_pool(name="dp", bufs=2))
        yp = ctx.enter_context(tc.tile_pool(name="yp", bufs=3))
        ps = ctx.enter_context(tc.tile_pool(name="ps", bufs=1, space="PSUM"))

        wq_sb = wp.tile([128, 16, DC], BF)
        wk_sb = wp.tile([128, 16, DC], BF)
        wv_sb = wp.tile([128, 16, DC], BF)
        wo_sb = wp.tile([128, HPC, E], BF)
        bq_sb = wp.tile([128, HPC, 1], F32)
        bk_sb = wp.tile([128, HPC, 1], F32)
        msk_sb = wp.tile([128, 4, 512], BF)
        onk_sb = wp.tile([128, 128], BF)

        for b in range(B):
            s0 = b * S
            x_sb = bp.tile([128, 16, S], BF)
            for t in range(16):
                nc.sync.dma_start(x_sb[:, t, :], xT[t * 128 : (t + 1) * 128, s0 : s0 + S])
                if b == 0:
                    # interleave weight loads with batch-0 x so the first
                    # projection chain starts as soon as its t-slice lands
                    nc.sync.dma_start(wq_sb[:, t, :], wqT[t * 128 : (t + 1) * 128, :])
                    nc.sync.dma_start(wk_sb[:, t, :], wkT[t * 128 : (t + 1) * 128, :])
            if b == 0:
                for h in range(HPC):
                    nc.sync.dma_start(bq_sb[:, h, :], bqd[h * 128 : (h + 1) * 128, :])
                    nc.sync.dma_start(bk_sb[:, h, :], bkd[h * 128 : (h + 1) * 128, :])
                for t in range(16):
                    nc.sync.dma_start(wv_sb[:, t, :], wvT[t * 128 : (t + 1) * 128, :])
                for r in range(4):
                    nc.sync.dma_start(msk_sb[:, r, :], mskd[r * 128 : (r + 1) * 128, :])
                nc.sync.dma_start(onk_sb[:], onkd[:])
                for h in range(HPC):
                    nc.sync.dma_start(wo_sb[:, h, :], woT[h * 128 : (h + 1) * 128, :])
            qT_sb = bp.tile([128, HPC, S], BF)
            kT_sb = bp.tile([128, HPC, S], BF)
            v_sb = bp.tile([128, 16, DC], BF)
            ctxN_sb = bp.tile([128, HPC, S], BF)

            # --- q/k/v projections (contract E in 16 chunks of 128) ---
            for m in range(HPC):
                for j in range(4):
                    js = slice(j * 512, (j + 1) * 512)
                    q_ps = ps.tile([128, 512], F32, tag="pr", bufs=2)
                    for t in range(16):
                        nc.tensor.matmul(
                            q_ps[:],
                            wq_sb[:, t, m * 128 : (m + 1) * 128],
                            x_sb[:, t, js],
                            start=(t == 0),
                            stop=(t == 15),
                        )
                    nc.scalar.activation(
                        qT_sb[:, m, js], q_ps[:], AF.Identity, bias=bq_sb[:, m, :]
                    )
                    k_ps = ps.tile([128, 512], F32, tag="pr", bufs=2)
                    for t in range(16):
                        nc.tensor.matmul(
                            k_ps[:],
                            wk_sb[:, t, m * 128 : (m + 1) * 128],
                            x_sb[:, t, js],
                            start=(t == 0),
                            stop=(t == 15),
                        )
                    nc.scalar.activation(
                        kT_sb[:, m, js], k_ps[:], AF.Identity, bias=bk_sb[:, m, :]
                    )
            for si in range(16):
                v_ps = ps.tile([128, DC], F32, tag="pr", bufs=2)
                for t in range(16):
                    nc.tensor.matmul(
                        v_ps[:],
                        x_sb[:, t, si * 128 : (si + 1) * 128],
                        wv_sb[:, t, :],
                        start=(t == 0),
                        stop=(t == 15),
                    )
                nc.vector.tensor_copy(v_sb[:, si, :], v_ps[:])

            # --- causal attention, scores kept transposed [k, q] ---
            for h in range(HPC):
                hd = slice(h * 128, (h + 1) * 128)
                for qb in range(4):
                    qs = slice(qb * 512, (qb + 1) * 512)
                    kmax = 4 * qb + 4
                    pts = []
                    for kc in range(kmax):
                        sc_ps = ps.tile([128, 512], F32, tag="sc", bufs=3)
                        diag = kc - 4 * qb
                        nc.tensor.matmul(
                            sc_ps[:],
                            kT_sb[:, h, kc * 128 : (kc + 1) * 128],
                            qT_sb[:, h, qs],
                            start=True,
                            stop=True,
                        )
                        p_t = pp.tile([128, 512], BF)
                        nc.scalar.activation(p_t[:], sc_ps[:], AF.Exp)
                        if diag >= 0:
                            nc.vector.tensor_tensor(
                                p_t[:], p_t[:], msk_sb[:, diag, :], ALU.mult
                            )
                        pts.append(p_t)
                    den_ps = ps.tile([128, 512], F32, tag="dn", bufs=1)
                    for kc in range(kmax):
                        nc.tensor.matmul(
                            den_ps[:],
                            onk_sb[:],
                            pts[kc][:],
                            start=(kc == 0),
                            stop=(kc == kmax - 1),
                        )
                    ctx_ps = ps.tile([128, 512], F32, tag="cx", bufs=2)
                    for kc in range(kmax):
                        nc.tensor.matmul(
                            ctx_ps[:],
                            v_sb[:, kc, hd],
                            pts[kc][:],
                            start=(kc == 0),
                            stop=(kc == kmax - 1),
                        )
                    lnd_sb = dp.tile([128, 512], F32, tag="lnd", bufs=2)
                    nc.scalar.activation(lnd_sb[:], den_ps[:], AF.Ln)
                    recb_sb = dp.tile([128, 512], F32, tag="recb", bufs=2)
                    nc.scalar.activation(recb_sb[:], lnd_sb[:], AF.Exp, scale=-1.0)
                    nc.vector.tensor_tensor(
                        ctxN_sb[:, h, qs], ctx_ps[:], recb_sb[:], ALU.mult
                    )

            # --- output projection (contract per-core d=256 in 2 head chunks) ---
            for qc in range(16):
                for eb in range(4):
                    y_ps = ps.tile([128, 512], F32, tag="pr", bufs=2)
                    nc.tensor.matmul(
                        y_ps[:],
                        ctxN_sb[:, 0, qc * 128 : (qc + 1) * 128],
                        wo_sb[:, 0, eb * 512 : (eb + 1) * 512],
                        start=True,
                        stop=False,
                    )
                    nc.tensor.matmul(
                        y_ps[:],
                        ctxN_sb[:, 1, qc * 128 : (qc + 1) * 128],
                        wo_sb[:, 1, eb * 512 : (eb + 1) * 512],
                        start=False,
                        stop=True,
                    )
                    y_t = yp.tile([128, 512], F32)
                    if (qc * 4 + eb) % 2 == 0:
                        nc.vector.tensor_copy(y_t[:], y_ps[:])
                    else:
                        nc.scalar.copy(y_t[:], y_ps[:])
                    nc.sync.dma_start(
                        yd[s0 + qc * 128 : s0 + (qc + 1) * 128, eb * 512 : (eb + 1) * 512],
                        y_t[:],
                    )
    _split_excess_waits(nc)
    return nc


def _mask_np():
    m = np.zeros((4, 128, 512), np.float32)
    kk = np.arange(128)[:, None]
    qq = np.arange(512)[None, :]
    for r in range(4):
        m[r] = np.where(kk + 128 * r > qq, 0.0, 1.0)
    return m.reshape(512, 512).astype(bf16)


def kernel(**inputs):
    global LAST_RESULTS, _NC_CACHE
    x = np.asarray(inputs["x"], np.float32)
    Wq = np.asarray(inputs["Wq"], np.float32)
    bq = np.asarray(inputs["bq"], np.float32)
    Wk = np.asarray(inputs["Wk"], np.float32)
    bk = np.asarray(inputs["bk"], np.float32)
    Wv = np.asarray(inputs["Wv"], np.float32)
    bv = np.asarray(inputs["bv"], np.float32)
    Wo = np.asarray(inputs["Wo"], np.float32)
    bo = np.asarray(inputs["bo"], np.float32)

    xT = np.ascontiguousarray(x.reshape(BS, E).T).astype(bf16)
    mask = _mask_np()
    onesk = np.ones((128, 128), bf16)

    in_maps = []
    for c in range(NCORES):
        dsl = slice(c * DC, (c + 1) * DC)
        in_maps.append(
            {
                "xT": xT,
                "wqT": np.ascontiguousarray(Wq[dsl].T * SCALE).astype(bf16),
                "wkT": np.ascontiguousarray(Wk[dsl].T).astype(bf16),
                "wvT": np.ascontiguousarray(Wv[dsl].T).astype(bf16),
                "woT": np.ascontiguousarray(Wo[:, dsl].T).astype(bf16),
                "bq": np.ascontiguousarray((bq[dsl] * SCALE).reshape(DC, 1)),
                "bk": np.ascontiguousarray(bk[dsl].reshape(DC, 1)),
                "mask": mask,
                "onesk": onesk,
            }
        )

    if _NC_CACHE is None:
        _NC_CACHE = _build()
    res = run_bass_kernel_spmd(_NC_CACHE, in_maps, core_ids=list(range(NCORES)), trace=TRACE)
    LAST_RESULTS = res

    acc = None
    for r in res.results:
        yc = np.asarray(r["y"], np.float32)
        acc = yc if acc is None else acc + yc
    bo_eff = bo + bv @ Wo.T
    acc += bo_eff[None, :]
    return acc.reshape(B, S, E).astype(np.float32)
